# revision 2
# baseline (speedup 1.0000x reference)
"""DiffLogicLayer forward on 8 TRN2 NeuronCores — gate-sharded, uint8 I/O.

Math: every one of the 16 soft logic ops is affine in {1, a, b, a*b}, so
    out[n, o] = C0[o] + C1[o]*a + C2[o]*b + C3[o]*a*b
with a = x[n, conn_a[o]], b = x[n, conn_b[o]] and C = softmax(weights) @ M
for the constant 16x4 matrix M of op coefficients (host-precomputed).

Quantization (the kernel is HBM-bandwidth bound, so bytes == time):
  - inputs:  x in [0,1) -> a_u = round(255*x) as uint8 (abs err <= 1/510),
    halving gather traffic vs bf16 (8 MiB/core instead of 16).
  - output:  out in [0,1] exactly (softmax-convex combination of logic ops
    that each map [0,1]^2 -> [0,1]), so o_u = trunc(247*out + 4.5) as uint8
    (scale 247 + offset 4 leaves +-4 LSB of margin against float->u8
    wrap/saturation ambiguity); host decodes (o_u - 4)/247.  4 MiB/core.
  Folding the scales into the per-gate coefficients K:
    K3 = 247*C3/255^2, K2 = 247*C2/255, K1 = 247*C1/255, K0 = 247*C0 + 4.5
  gives 247*out + 4.5 = (K3*a_u + K2)*b_u + (K1*a_u + K0), computed per
  128-gate slot in exactly two fused DVE ops:
    p = affine_mul_reduce(a_u, b_u, K3, K2)   # (a*K3 + K2) * b
    o = affine_then_add (a_u, p,   K1, K0)    # (a*K1 + K0) + p -> uint8

Sharding: out_dim (gate axis) split 8 ways; each core owns 1024 gates and
the full 4096-batch. Host supplies xT = x.T quantized to uint8 so each
gathered operand row is 4 KiB contiguous. Gathers move 512 gates per
dma_gather (4 calls/core instead of 16 — SWDGE fixed overhead is ~1 us per
call), a-gathers on SWDGE queue 0 and b-gathers on queue 1.
"""

import numpy as np
from contextlib import ExitStack

import concourse.bacc as bacc
import concourse.mybir as mybir
import concourse.tile as tile
from concourse.bass_utils import run_bass_kernel_spmd

N_CORES = 8
BATCH, IN_DIM, OUT_DIM = 4096, 4096, 8192
GPC = OUT_DIM // N_CORES          # gates per core = 1024
SLOTS = GPC // 128                # 128-gate compute slots per core = 8
GIDX = 512                        # gates per dma_gather
NGATHER = GPC // GIDX             # gathers per operand per core = 2
BLKS = GIDX // 128                # compute slots per gather = 4
F32 = mybir.dt.float32
BF16 = mybir.dt.bfloat16
U8 = mybir.dt.uint8
I16 = mybir.dt.int16

OUT_SCALE = 247.0                 # uint8 output: o = trunc(247*out + 4.5)
OUT_OFF = 4.0

# coefficient matrix: op i -> (c0, c1, c2, c3) with value c0 + c1*a + c2*b
# + c3*a*b; rows follow the reference's 16-op ordering.
_OP2AFF = np.array([
    [0, 0, 0, 0],     # false
    [0, 0, 0, 1],     # a and b
    [0, 1, 0, -1],    # a and not b
    [0, 1, 0, 0],     # a
    [0, 0, 1, -1],    # not a and b
    [0, 0, 1, 0],     # b
    [0, 1, 1, -2],    # xor
    [0, 1, 1, -1],    # or
    [1, -1, -1, 1],   # nor
    [1, -1, -1, 2],   # xnor
    [1, 0, -1, 0],    # not b
    [1, 0, -1, 1],    # a or not b
    [1, -1, 0, 0],    # not a
    [1, -1, 0, 1],    # not a or b
    [1, 0, 0, -1],    # nand
    [1, 0, 0, 0],     # true
], dtype=np.float32)

_compiled = {}


def _build_nc(reps=1):
    """Build the per-core program. `reps` unrolls the whole kernel body
    that many times (all reps recompute the identical full output) —
    used by the timing harness to amortize per-dispatch overhead; the
    functional kernel() path uses reps=1."""
    nc = bacc.Bacc("TRN2", target_bir_lowering=False, debug=False,
                   num_devices=N_CORES, num_swdge_queues=2)
    xT = nc.dram_tensor("xT", [IN_DIM, BATCH], U8, kind="ExternalInput")
    iw = GIDX // 16                   # idx free-cols per gather
    ia_d = nc.dram_tensor("ia", [128, NGATHER * iw], I16, kind="ExternalInput")
    ib_d = nc.dram_tensor("ib", [128, NGATHER * iw], I16, kind="ExternalInput")
    cf_d = nc.dram_tensor("cf", [4, 128, SLOTS], F32, kind="ExternalInput")
    outT = nc.dram_tensor("outT", [GPC, BATCH], U8, kind="ExternalOutput")

    with tile.TileContext(nc) as tc, ExitStack() as ctx:
        const = ctx.enter_context(tc.tile_pool(name="const", bufs=1))
        pa = ctx.enter_context(tc.tile_pool(name="a", bufs=2))
        pb = ctx.enter_context(tc.tile_pool(name="b", bufs=2))
        pp = ctx.enter_context(tc.tile_pool(name="p", bufs=2))
        po = ctx.enter_context(tc.tile_pool(name="o", bufs=3))
        pacc = ctx.enter_context(tc.tile_pool(name="acc", bufs=4))

        # index tiles first: the gathers depend on them, the coefficient
        # tiles are only needed once compute starts.
        ia = const.tile([128, NGATHER * iw], I16, tag="ia")
        ib = const.tile([128, NGATHER * iw], I16, tag="ib")
        nc.sync.dma_start(ia[:], ia_d.ap()[:])
        nc.sync.dma_start(ib[:], ib_d.ap()[:])
        ks = []
        for k in range(4):
            ck = const.tile([128, SLOTS], F32, tag=f"k{k}")
            nc.sync.dma_start(ck[:], cf_d.ap()[k])
            ks.append(ck)
        K0, K1, K2, K3 = ks

        for rep in range(reps):
            for g in range(NGATHER):
                A = pa.tile([128, BLKS, BATCH], U8, tag="A")
                nc.gpsimd.dma_gather(A[:], xT.ap()[:],
                                     ia[:, g * iw:(g + 1) * iw],
                                     GIDX, GIDX, BATCH, queue_num=0)
                B = pb.tile([128, BLKS, BATCH], U8, tag="B")
                nc.gpsimd.dma_gather(B[:], xT.ap()[:],
                                     ib[:, g * iw:(g + 1) * iw],
                                     GIDX, GIDX, BATCH, queue_num=1)
                for blk in range(BLKS):
                    s = g * BLKS + blk
                    a2, b2 = A[:, blk, :], B[:, blk, :]
                    p = pp.tile([128, BATCH], BF16, tag="p")
                    acc = pacc.tile([128, 1], F32, tag="acc")
                    nc.vector.affine_mul_reduce(p[:], acc[:], a2, b2,
                                                K3[:, s:s + 1], K2[:, s:s + 1])
                    o = po.tile([128, BATCH], U8, tag="o")
                    nc.vector.affine_then_add(o[:], a2, p[:],
                                              K1[:, s:s + 1], K0[:, s:s + 1])
                    nc.sync.dma_start(outT.ap()[s * 128:(s + 1) * 128, :], o[:])

    nc.compile()
    return nc


def _wrap_idx(conn_shard: np.ndarray) -> np.ndarray:
    """SWDGE index wrapping: per gather of GIDX gates, list position i sits
    at partition i%16, free slot i//16, replicated across the 8 Q7 core
    partition blocks of 16."""
    iw = GIDX // 16
    w = np.empty((128, NGATHER * iw), np.int16)
    for gc in range(NGATHER):
        blk = conn_shard[gc * GIDX:(gc + 1) * GIDX].reshape(iw, 16).T
        w[:, gc * iw:(gc + 1) * iw] = np.tile(blk, (8, 1))
    return w


def make_in_maps(x, weights, conn_a, conn_b):
    x = np.asarray(x, dtype=np.float32)
    weights = np.asarray(weights, dtype=np.float32)
    ca = np.asarray(conn_a).astype(np.int16)
    cb = np.asarray(conn_b).astype(np.int16)
    # softmax(weights) @ affine-coefficient matrix -> [OUT_DIM, 4] f32
    e = np.exp(weights - weights.max(axis=1, keepdims=True))
    sm = e / e.sum(axis=1, keepdims=True)
    cofs = sm @ _OP2AFF                                  # [OUT_DIM, 4]
    # fold the u8 in/out scales into the coefficients (see module docstring)
    kmat = np.empty_like(cofs)                           # [OUT_DIM, 4]
    kmat[:, 0] = OUT_SCALE * cofs[:, 0] + OUT_OFF + 0.5
    kmat[:, 1] = OUT_SCALE * cofs[:, 1] / 255.0
    kmat[:, 2] = OUT_SCALE * cofs[:, 2] / 255.0
    kmat[:, 3] = OUT_SCALE * cofs[:, 3] / (255.0 * 255.0)
    xT = np.ascontiguousarray(
        np.rint(x.T * 255.0).astype(np.uint8))           # [4096, 4096] u8
    in_maps = []
    perms = []
    for c in range(N_CORES):
        g0, g1 = c * GPC, (c + 1) * GPC
        # sort this core's gates by conn_a: ascending gather addresses are
        # HBM-friendlier; pure host-side permutation, undone in assemble_out
        perm = np.argsort(ca[g0:g1], kind="stable")
        perms.append(perm)
        # position g0 + 128*s + p holds gate perm[128*s + p] -> cf[k, p, s]
        cf = np.ascontiguousarray(
            kmat[g0:g1][perm].reshape(SLOTS, 128, 4).transpose(2, 1, 0))
        in_maps.append({
            "xT": xT,
            "ia": _wrap_idx(ca[g0:g1][perm]),
            "ib": _wrap_idx(cb[g0:g1][perm]),
            "cf": cf.astype(np.float32),
        })
    _compiled["perms"] = perms
    return in_maps


def get_nc(reps=1):
    key = ("nc", reps)
    if key not in _compiled:
        _compiled[key] = _build_nc(reps)
    return _compiled[key]


def assemble_out(results) -> np.ndarray:
    perms = _compiled["perms"]
    out = np.empty((BATCH, OUT_DIM), np.float32)
    inv = 1.0 / OUT_SCALE
    for c in range(N_CORES):
        arr = np.asarray(results[c]["outT"])             # [1024, 4096] u8
        dec = (arr.T.astype(np.float32) - OUT_OFF) * inv
        out[:, c * GPC + perms[c]] = dec
    return out


def kernel(x, weights, conn_a, conn_b) -> np.ndarray:
    nc = get_nc()
    in_maps = make_in_maps(x, weights, conn_a, conn_b)
    res = run_bass_kernel_spmd(nc, in_maps, core_ids=list(range(N_CORES)))
    return assemble_out(res.results)
